# revision 63
# baseline (speedup 1.0000x reference)
"""Trainium2 Bass kernel for GQA attention prefill (B=2, T=2048, D=4096, N=32, K=8, H=128).

Sharding: 8 cores = 2 (batch) x 4 (head-groups). Each core handles one batch
element, 8 q-heads and its 2 kv-heads, producing a partial output projection
(summed over its heads). Host sums the 4 partials per batch element (and
undoes the x512 weight scaling).

Precision scheme (PE cost model: bf16/fp16 1.0 cycles/row, fp8+DoubleRow 0.5
cycles/row with a 256-deep contraction -> 4x effective throughput):
  - q/k/v/o projections run as fp8 DoubleRow with hi+lo error compensation:
    w ~ whi + wlo, x ~ xhi + xlo (each e4m3), y = whi@xhi + wlo@xhi + whi@xlo.
    3 quarter-cost matmuls = 0.75x the bf16 cost at ~0.1% error. Weights are
    pre-scaled into e4m3's normal range (wq,wk x64 folded into the exp scale;
    wv x16 cancels against the softmax 1/l fold; wo x32 undone on host).
  - one o-proj head-pair runs direct fp8 (1 matmul, 0.25x cost), spending the
    correctness headroom (~1.7% of final norm).
  - attention (rope, logits, exp, AV) runs in fp16: same PE cost as bf16,
    ~8x lower noise.

Per-core pipeline, software-pipelined per head so PE never idles:
  passA(tb):  k,v projections from xhi/xlo (DMA'd once per t-block, resident
              in SBUF); rope(k) via SBUF->SBUF DMA half-swap plus DVE
              elementwise with fp16 cos/sin tables ([-sin; sin] fold).
  per head h: q-projection matmuls for head h+2 are emitted interleaved with
              head h's attention s-block loop. Attention: logitsT [s128,t<=512]
              = kT-block @ qt (fp16), exp on ACT (scale absorbs the x64 weight
              scales), 0/1 triangle mask multiply on DVE for diagonal tiles,
              AV accumulates in PSUM; denominators accumulate on DVE in f32.
  fin(h):     gpsimd partition reduce -> reciprocal -> DVE psum*rinv -> f32
              tmp, then ACT copy -> enc_hi (fp8) and DVE sub -> enc_lo (fp8),
              pair-interleaved for the o-proj stationary operand.
  ph3(tb):    output projection from enc pair tiles: 3 pairs x 3-term + 1
              direct pair = 10 DoubleRow matmuls per (dchunk, tchunk); PSUM ->
              bf16 SBUF copies on ACT, DMA out per 512-wide d-chunk.
"""

import os
import sys

import numpy as np

for _p in ("/opt/trn_rl_repo", "/root/.axon_site/_ro/trn_rl_repo"):
    if _p not in sys.path and os.path.isdir(_p):
        sys.path.append(_p)

import ml_dtypes

BF16 = ml_dtypes.bfloat16
F16 = np.float16
F8 = ml_dtypes.float8_e4m3fn

P = 128
T = 2048
D = 4096
H = 128
NQ = 8   # q heads per core
NKV = 2  # kv heads per core
TB = 512
NTB = T // TB        # 4
DT = D // P          # 32 d-tiles
NDP = DT // 2        # 16 d-tile pairs
NSB = T // P         # 16 s-blocks
TC = TB // P         # 4 t-chunks per t-block
NDC = D // TB        # 8 d-chunks for the output projection
SCALE = float(H) ** -0.5
SW_QK = 64.0         # wq/wk host scale (folded into exp scale)
SW_V = 16.0          # wv host scale (cancels vs softmax 1/l fold)
SW_O = 32.0          # wo host scale (undone on host with 1/(SW_V*SW_O))
EXP_SCALE = SCALE / (SW_QK * SW_QK)
DIRECT_PAIR = 3      # o-proj head pair computed hi@hi only ...
DIRECT_DCS = frozenset({0, 1, 2, 3, 4, 5, 6})  # ... on these 512-wide d-chunks

_STATE = {}


def _build_nc():
    import concourse.mybir as mybir
    import concourse.tile as tile
    from concourse import bacc
    from concourse import bass_isa

    f32 = mybir.dt.float32
    fp16 = mybir.dt.float16
    fp8 = mybir.dt.float8e4
    bf16 = mybir.dt.bfloat16
    Alu = mybir.AluOpType
    Act = mybir.ActivationFunctionType
    DR = mybir.MatmulPerfMode.DoubleRow

    nc = bacc.Bacc(None, target_bir_lowering=False, debug=False)

    xhi = nc.dram_tensor("xhi", [D, T], fp8, kind="ExternalInput")
    xlo = nc.dram_tensor("xlo", [D, T], fp8, kind="ExternalInput")
    # weights are partition-major and hi/lo-packed so each load is one
    # fully-contiguous DMA (>=512B runs avoid the half-bandwidth penalty):
    # wq8[h, p] = [NDP, 2(hi/lo), 2(pair), H], wk8/wv8[p] likewise
    wq8 = nc.dram_tensor("wq8", [NQ, P, 2, NDP, 2, H], fp8,
                         kind="ExternalInput")
    wk8 = nc.dram_tensor("wk8", [P, 2, NDP, 2, NKV, H], fp8,
                         kind="ExternalInput")
    wv8 = nc.dram_tensor("wv8", [P, 2, NDP, 2, NKV * H], fp8,
                         kind="ExternalInput")
    # o-projection weights, head-major within rows: [H, NQ, D] (adjacent
    # heads form the DoubleRow pairs)
    woh = nc.dram_tensor("woh", [H, NQ, D], fp8, kind="ExternalInput")
    wol = nc.dram_tensor("wol", [H, NQ, D], fp8, kind="ExternalInput")
    cos = nc.dram_tensor("cos", [P, T], fp16, kind="ExternalInput")
    sin = nc.dram_tensor("sin", [P, T], fp16, kind="ExternalInput")
    tri = nc.dram_tensor("tri", [P, P], fp16, kind="ExternalInput")
    y = nc.dram_tensor("y", [T, D], bf16, kind="ExternalOutput")

    with tile.TileContext(nc) as tc:
        with (
            tc.tile_pool(name="const", bufs=1) as const,
            tc.tile_pool(name="xp", bufs=2) as xp,
            tc.tile_pool(name="wqp", bufs=3) as wqp,
            tc.tile_pool(name="qtp", bufs=3) as qtp,
            tc.tile_pool(name="rp", bufs=2) as rp,
            tc.tile_pool(name="ep", bufs=6) as ep,
            tc.tile_pool(name="eap", bufs=2) as eap,
            tc.tile_pool(name="encp", bufs=1) as encp,
            tc.tile_pool(name="lp", bufs=1) as lp,
            tc.tile_pool(name="wop", bufs=2) as wop,
            tc.tile_pool(name="yp", bufs=2) as yp,
            tc.tile_pool(name="ps", bufs=1, space="PSUM") as ps,
        ):
            wk_t = const.tile([P, 2, NDP, 2, NKV, H], fp8, tag="wk")
            wv_t = const.tile([P, 2, NDP, 2, NKV * H], fp8, tag="wv")
            tri_sb = const.tile([P, P], fp16, tag="tri")
            cos_sb = const.tile([P, 2, TB], fp16, tag="cos")
            sin_sb = const.tile([P, 2, TB], fp16, tag="sin")
            kT_all = const.tile([P, NKV, T], fp16, tag="kT")
            v_all = const.tile([P, NKV, NSB, H], fp16, tag="v")

            def x_dma_closures(tb, x_hi, x_lo, tables=True, nch=8):
                """nch closures, each DMA-ing a DT/nch-d-tile chunk of
                xhi+xlo for tb (plus this t-block's cos/sin on chunk 0)."""
                tsl = slice(tb * TB, (tb + 1) * TB)
                step = DT // nch

                def mk(c8):
                    def emit():
                        dsl = slice(c8 * step * P, (c8 + 1) * step * P)
                        csl = slice(c8 * step, (c8 + 1) * step)
                        nc.sync.dma_start(
                            x_hi[:, csl, :],
                            xhi[dsl, tsl].rearrange("(g p) t -> p g t", p=P))
                        nc.sync.dma_start(
                            x_lo[:, csl, :],
                            xlo[dsl, tsl].rearrange("(g p) t -> p g t", p=P))
                        if c8 == 0 and tables:
                            nc.sync.dma_start(cos_sb[:, tb % 2, :],
                                              cos[:, tsl])
                            nc.sync.dma_start(sin_sb[:, tb % 2, :],
                                              sin[:, tsl])
                    return emit

                return [mk(c8) for c8 in range(nch)]

            def mm3(out, wf, mf, pr, start, stop):
                """3-term compensated DoubleRow accumulation for d-pair pr.
                wf(pr, s) -> [P, 2, M] stationary slice, mf(pr, s) ->
                [P, 2, N] moving slice (s: 0=hi, 1=lo)."""
                nc.tensor.matmul(out, wf(pr, 0), mf(pr, 0),
                                 start=start, stop=False, perf_mode=DR)
                nc.tensor.matmul(out, wf(pr, 1), mf(pr, 0),
                                 start=False, stop=False, perf_mode=DR)
                nc.tensor.matmul(out, wf(pr, 0), mf(pr, 1),
                                 start=False, stop=stop, perf_mode=DR)

            def rope(dst, src_ps, tb):
                """dst[:] = rope(src_ps) for one head's [H, TB] block (fp16).
                Half-swap via PSUM->SBUF DMA partition reorder; the sign of
                the swapped half is folded into the sin table ([-sin; +sin])."""
                cs = cos_sb[:, tb % 2, :]
                sn = sin_sb[:, tb % 2, :]
                raw = rp.tile([P, TB], fp16, tag="raw", name="raw")
                nc.scalar.copy(raw[:], src_ps[:])
                shuf = rp.tile([P, TB], fp16, tag="shuf", name="shuf")
                nc.sync.dma_start(shuf[0:P // 2, :], raw[P // 2:P, :])
                nc.sync.dma_start(shuf[P // 2:P, :], raw[0:P // 2, :])
                tmp = rp.tile([P, TB], fp16, tag="tmp", name="tmp")
                nc.vector.tensor_tensor(dst, raw[:], cs, Alu.mult)
                nc.vector.tensor_tensor(tmp[:], shuf[:], sn, Alu.mult)
                nc.vector.tensor_tensor(dst, dst, tmp[:], Alu.add)

            def kv_k_chunks(tbx, split=False):
                """k-projection chunk closures (+ psk tiles) for tbx. With
                split=True, returns hi-closures (hi@hi terms only) followed
                by lo-closures (compensation terms), so the prologue can
                start on the hi DMAs alone."""
                x_hi, x_lo = x_tiles[tbx]
                psk = [ps.tile([P, TB], f32, tag="big", bufs=6,
                               name=f"psk{_k}") for _k in range(NKV)]

                def xf(pr, s):
                    return (x_hi if s == 0 else x_lo)[:, 2 * pr:2 * pr + 2, :]

                def wf(kk):
                    return lambda pr, s: wk_t[:, s, pr, :, kk, :]

                def mk(p0, p1):
                    def emit():
                        for pr in range(p0, p1):
                            for kk in range(NKV):
                                mm3(psk[kk][:], wf(kk), xf, pr,
                                    start=pr == 0, stop=pr == NDP - 1)
                    return emit

                def mk_hi(p0, p1):
                    def emit():
                        for pr in range(p0, p1):
                            for kk in range(NKV):
                                nc.tensor.matmul(
                                    psk[kk][:], wk_t[:, 0, pr, :, kk, :],
                                    xf(pr, 0), start=pr == 0,
                                    stop=False, perf_mode=DR)
                    return emit

                def mk_lo(p0, p1):
                    def emit():
                        for pr in range(p0, p1):
                            for kk in range(NKV):
                                nc.tensor.matmul(
                                    psk[kk][:], wk_t[:, 1, pr, :, kk, :],
                                    xf(pr, 0), start=False,
                                    stop=False, perf_mode=DR)
                                nc.tensor.matmul(
                                    psk[kk][:], wk_t[:, 0, pr, :, kk, :],
                                    xf(pr, 1), start=False,
                                    stop=pr == NDP - 1 and kk == NKV - 1,
                                    perf_mode=DR)
                    return emit

                if split:
                    cls = ([mk_hi(p, p + 2) for p in range(0, NDP, 2)] +
                           [mk_lo(p, p + 2) for p in range(0, NDP, 2)])
                    return cls, psk
                return [mk(p, p + 2) for p in range(0, NDP, 2)], psk

            def kv_v_chunks(tbx):
                """v-projection closures for tbx. Each of the 4 t-chunks gets
                its OWN PSUM tile, sequenced so a chunk is copied out to
                v_all before its bank slot is reused."""
                x_hi, x_lo = x_tiles[tbx]
                state = {}

                def mk_mm(c, p0, p1):
                    def emit():
                        if p0 == 0:
                            state[c] = ps.tile([P, NKV * H], f32, tag="big",
                                               bufs=6, name=f"psv{c}")
                        csl = slice(c * P, (c + 1) * P)

                        def xf(pr, s):
                            return (x_hi if s == 0
                                    else x_lo)[:, 2 * pr:2 * pr + 2, csl]

                        for pr in range(p0, p1):
                            mm3(state[c][:], xf,
                                lambda pr, s: wv_t[:, s, pr, :, :], pr,
                                start=pr == 0, stop=pr == NDP - 1)
                    return emit

                def mk_copy(c):
                    def emit():
                        nc.scalar.copy(
                            v_all[:, :, tbx * TC + c, :],
                            state[c][:].rearrange("p (h e) -> p h e", h=NKV))
                    return emit

                chunks = []
                for c in (0, 2):
                    chunks += [mk_mm(c, p, p + 4) for p in range(0, NDP, 4)]
                chunks.append(mk_copy(0))
                chunks += [mk_mm(1, p, p + 4) for p in range(0, NDP, 4)]
                chunks.append(mk_copy(2))
                chunks += [mk_mm(3, p, p + 4) for p in range(0, NDP, 4)]
                chunks.append(mk_copy(1))
                chunks.append(mk_copy(3))
                return chunks, None

            def kv_finish_k(tbx, psk):
                tsl = slice(tbx * TB, (tbx + 1) * TB)
                for kk in range(NKV):
                    rope(kT_all[:, kk, tsl], psk[kk], tbx)

            wqs = {}
            psqs = {}

            def load_wq(h, tb):
                wq_t = wqp.tile([P, 2, NDP, 2, H], fp8, tag="wq",
                                name=f"wq{h}")
                nc.sync.dma_start(wq_t[:], wq8[h])
                wqs[h] = wq_t

            def qchunks(h, tb, split=False):
                """Closures each emitting a few of head h's 48 accumulating
                q-projection DoubleRow matmuls (wq must already be loading)."""
                x_hi, x_lo = x_tiles[tb]
                wq_t = wqs[h]
                psq = ps.tile([P, TB], f32, tag="big", bufs=6, name=f"psq{h}")
                psqs[h] = psq

                def xf(pr, s):
                    return (x_hi if s == 0 else x_lo)[:, 2 * pr:2 * pr + 2, :]

                def mk(p0, p1):
                    def emit():
                        for pr in range(p0, p1):
                            mm3(psq[:], lambda pr, s: wq_t[:, s, pr, :, :],
                                xf, pr, start=pr == 0, stop=pr == NDP - 1)
                    return emit

                if split:
                    def mk_hi(p0, p1):
                        def emit():
                            for pr in range(p0, p1):
                                nc.tensor.matmul(
                                    psq[:], wq_t[:, 0, pr, :, :], xf(pr, 0),
                                    start=pr == 0, stop=False, perf_mode=DR)
                        return emit

                    def mk_lo(p0, p1):
                        def emit():
                            for pr in range(p0, p1):
                                nc.tensor.matmul(
                                    psq[:], wq_t[:, 1, pr, :, :], xf(pr, 0),
                                    start=False, stop=False, perf_mode=DR)
                                nc.tensor.matmul(
                                    psq[:], wq_t[:, 0, pr, :, :], xf(pr, 1),
                                    start=False, stop=pr == NDP - 1,
                                    perf_mode=DR)
                        return emit

                    return ([mk_hi(p, p + 2) for p in range(0, NDP, 2)],
                            [mk_lo(p, p + 2) for p in range(0, NDP, 2)])
                return [mk(p, p + 2) for p in range(0, NDP, 2)]

            def attn(h, tb, qt, hp, fills, tail_fn, pe_fills=True,
                     head_fn=None, fin_prev=None):
                """Attention for head h. `fills` are closures interleaved into
                the s-block loop (a later head's q matmuls, or DMA prefetch);
                `tail_fn` (that head's rope) is emitted once fills are done."""
                nsb = TC * (tb + 1)
                ndg = nsb - TC  # non-diagonal s-block count
                kk = h // 4
                enc_ps = ps.tile([P, TB], f32, tag="acc", bufs=2,
                                 name=f"encps{h}")
                exacc = eap.tile([P, TB], fp16, tag="eacc", name=f"eacc{h}")
                look = 5
                ex = [None] * nsb
                tail_state = {"done": tail_fn is None}

                def pre(sb):
                    r = sb - ndg
                    off = P * r if r >= 0 else 0
                    csl = slice(off, TB)
                    lg = ps.tile([P, TB], f32, tag="big", bufs=6,
                                 name=f"lg{sb}")
                    nc.tensor.matmul(lg[:, csl],
                                     kT_all[:, kk, sb * P:(sb + 1) * P],
                                     qt[:, csl])
                    ex_t = ep.tile([P, TB], fp16, tag="ex", name=f"ex{sb}")
                    nc.scalar.activation(ex_t[:, csl], lg[:, csl], Act.Exp,
                                         scale=EXP_SCALE)
                    if r >= 0:
                        nc.vector.tensor_tensor(ex_t[:, off:off + P],
                                                ex_t[:, off:off + P],
                                                tri_sb[:], Alu.mult)
                    ex[sb] = (ex_t, csl)

                def post(sb):
                    ex_t, csl = ex[sb]
                    nc.tensor.matmul(enc_ps[:, csl], v_all[:, kk, sb, :],
                                     ex_t[:, csl],
                                     start=sb == 0, stop=sb == nsb - 1)
                    # exp-tile accumulation for the softmax denominator (DVE)
                    if sb == 0:
                        nc.vector.tensor_copy(exacc[:], ex_t[:])
                    else:
                        nc.vector.tensor_tensor(exacc[:, csl], exacc[:, csl],
                                                ex_t[:, csl], Alu.add)

                ci = 0
                while ci < min(2, len(fills)):
                    fills[ci]()
                    ci += 1
                if head_fn is not None:
                    head_fn()
                for sb in range(min(look, nsb)):
                    pre(sb)
                if fin_prev is not None:
                    fin_prev()
                for sb in range(nsb):
                    want = min(len(fills),
                               max((sb + 3) * len(fills) // nsb, 3))
                    while ci < want:
                        fills[ci]()
                        ci += 1
                    if ci == len(fills) and not tail_state["done"]:
                        tail_fn()
                        tail_state["done"] = True
                    post(sb)
                    if sb + look < nsb:
                        pre(sb + look)
                while ci < len(fills):
                    fills[ci]()
                    ci += 1
                if not tail_state["done"]:
                    tail_fn()

                def finalize():
                    lsum = lp.tile([P, TB], f32, tag="lsum", name="lsum")
                    nc.gpsimd.partition_all_reduce(lsum[:], exacc[:], P,
                                                   bass_isa.ReduceOp.add)
                    rinv = lp.tile([P, TB], f32, tag="rinv", name="rinv")
                    nc.vector.reciprocal(rinv[:], lsum[:])
                    tmp = lp.tile([P, TB], f32, tag="etmp", name="etmp")
                    nc.vector.tensor_tensor(tmp[:], enc_ps[:], rinv[:],
                                            Alu.mult)
                    pr, parity = h // 2, h % 2
                    hi_sl = enc_hi[pr][:, parity, :]
                    nc.vector.tensor_copy(hi_sl, tmp[:])
                    nc.vector.tensor_tensor(enc_lo[pr][:, parity, :],
                                            tmp[:], hi_sl, Alu.subtract)
                return finalize

            wo_tiles = {}

            def load_wo(dc):
                dsl = slice(dc * TB, (dc + 1) * TB)
                wo_h = wop.tile([P, NQ, TB], fp8, tag="woh", bufs=4,
                                name=f"woh{dc}")
                nc.sync.dma_start(wo_h[:], woh[:, :, dsl])
                wo_l = wop.tile([P, 6, TB], fp8, tag="wol", bufs=4,
                                name=f"wol{dc}")
                nc.sync.dma_start(wo_l[:], wol[:, 0:6, dsl])
                wo_l2 = None
                if dc not in DIRECT_DCS:
                    # pair 3's lo weights are only read on non-direct chunks
                    wo_l2 = wop.tile([P, 2, TB], fp8, tag="wol2", bufs=1,
                                     name=f"wol2_{dc}")
                    nc.sync.dma_start(wo_l2[:], wol[:, 6:8, dsl])
                wo_tiles[dc] = (wo_h, wo_l, wo_l2)

            def oproj_pair(yps, pr, dc, tci, start, stop):
                """o-projection matmuls for head pair pr into yps.
                3-term compensated, or single hi@hi for the direct pair on
                the direct d-chunks."""
                wo_h, wo_l, wo_l2 = wo_tiles[dc]
                tsl = slice(tci * P, (tci + 1) * P)
                psl = slice(2 * pr, 2 * pr + 2)
                e_hi = enc_hi[pr][:, :, tsl]
                direct = pr == DIRECT_PAIR and dc in DIRECT_DCS
                nc.tensor.matmul(yps, e_hi, wo_h[:, psl, :], start=start,
                                 stop=stop and direct, perf_mode=DR)
                if not direct:
                    lo_sl = (wo_l2[:, 0:2, :] if pr == DIRECT_PAIR
                             else wo_l[:, psl, :])
                    nc.tensor.matmul(yps, e_hi, lo_sl, start=False,
                                     stop=False, perf_mode=DR)
                    nc.tensor.matmul(yps, enc_lo[pr][:, :, tsl],
                                     wo_h[:, psl, :], start=False, stop=stop,
                                     perf_mode=DR)

            def ph3(tb, nxt, pp=None):
                for dc in range(NDC):
                    if nxt is not None and dc == 4:
                        load_wq(3, nxt)
                    if dc + 4 < NDC:
                        load_wo(dc + 4)
                    # On dc 0, pair 3 (DIRECT_PAIR, whose enc depends on the
                    # final fin chain) is deferred behind the other pairs so
                    # fin's latency hides behind PE work; later dcs complete
                    # each t-chunk immediately so its PSUM bank frees early.
                    def finish(tci, yps):
                        oproj_pair(yps[:], DIRECT_PAIR, dc, tci,
                                   start=False, stop=True)
                        ys = yp.tile([P, TB], bf16, tag="ys", bufs=5,
                                     name=f"ys{dc}_{tci}")
                        if (dc * TC + tci) % 2 == 0:
                            nc.scalar.copy(ys[:], yps[:])
                        else:
                            nc.vector.tensor_copy(ys[:], yps[:])
                        nc.sync.dma_start(
                            y[tb * TB + tci * P:tb * TB + (tci + 1) * P,
                              dc * TB:(dc + 1) * TB], ys[:])

                    ytiles = {}
                    for tci in range(TC):
                        if pp and dc == 0 and tci in pp:
                            ytiles[tci] = pp[tci]  # pairs 0..2 already done
                            continue
                        yps = ps.tile([P, TB], f32, tag="big", bufs=6,
                                      name=f"yps{dc}_{tci}")
                        ytiles[tci] = yps
                        for pr in range(3):
                            oproj_pair(yps[:], pr, dc, tci,
                                       start=pr == 0, stop=False)
                        if dc != 0:
                            finish(tci, yps)
                    if dc == 0:
                        for tci in range(TC):
                            finish(tci, ytiles[tci])

            # ---- startup: hi-phase first (k hi@hi streams on the hi DMAs
            # alone), then lo/v phase ----
            xh0 = xp.tile([P, DT, TB], fp8, tag="xh", name="xh0")
            xl0 = xp.tile([P, DT, TB], fp8, tag="xl", name="xl0")
            x_tiles = {0: (xh0, xl0)}

            def _xdma(dst, src, g):
                nc.sync.dma_start(
                    dst[:, 4 * g:4 * (g + 1), :],
                    src[4 * g * P:4 * (g + 1) * P, 0:TB]
                    .rearrange("(g p) t -> p g t", p=P))

            kcs, psk0 = kv_k_chunks(0, split=True)  # 8 hi + 8 lo closures

            # v-projection runs as two PSUM groups in the prologue: the hi
            # group completes during the hi-DMA phase (PE fill), the lo group
            # in phase 2, DVE-added into v_all afterwards.
            vhi_state = {}
            vlo_state = {}

            def v_hi_chunk(g):
                def emit():
                    for c in range(TC):
                        if g == 0:
                            vhi_state[c] = ps.tile([P, NKV * H], f32,
                                                   tag="big", bufs=6,
                                                   name=f"psvh{c}")
                        csl = slice(c * P, (c + 1) * P)
                        for pr in (2 * g, 2 * g + 1):
                            nc.tensor.matmul(
                                vhi_state[c][:],
                                xh0[:, 2 * pr:2 * pr + 2, csl],
                                wv_t[:, 0, pr, :, :],
                                start=pr == 0, stop=pr == NDP - 1,
                                perf_mode=DR)
                def v_hi_copy():
                    for c in range(TC):
                        nc.scalar.copy(
                            v_all[:, :, c, :],
                            vhi_state[c][:]
                            .rearrange("p (h e) -> p h e", h=NKV))
                return emit if g < 8 else v_hi_copy

            def v_lo(c):
                vlo_state[c] = ps.tile([P, NKV * H], f32, tag="big", bufs=6,
                                       name=f"psvl{c}")
                csl = slice(c * P, (c + 1) * P)
                for pr in range(NDP):
                    nc.tensor.matmul(vlo_state[c][:],
                                     xl0[:, 2 * pr:2 * pr + 2, csl],
                                     wv_t[:, 0, pr, :, :],
                                     start=pr == 0, stop=False,
                                     perf_mode=DR)
                    nc.tensor.matmul(vlo_state[c][:],
                                     xh0[:, 2 * pr:2 * pr + 2, csl],
                                     wv_t[:, 1, pr, :, :],
                                     start=False, stop=pr == NDP - 1,
                                     perf_mode=DR)
                nc.vector.tensor_tensor(
                    v_all[:, :, c, :], v_all[:, :, c, :],
                    vlo_state[c][:].rearrange("p (h e) -> p h e", h=NKV),
                    Alu.add)

            # phase 1: hi halves of wk/wv (first pair-chunk split off so PE
            # can start early) + x-hi chunks, with k-hi and v-hi matmuls
            # pacing behind the chunks
            nc.sync.dma_start(wk_t[:, 0, 0:2], wk8[:, 0, 0:2])
            _xdma(xh0, xhi, 0)
            nc.sync.dma_start(wv_t[:, 0, 0:2], wv8[:, 0, 0:2])
            nc.sync.dma_start(wk_t[:, 0, 2:], wk8[:, 0, 2:])
            nc.sync.dma_start(wv_t[:, 0, 2:], wv8[:, 0, 2:])
            for g in range(1, 8):
                _xdma(xh0, xhi, g)
                kcs[g - 1]()
                v_hi_chunk(g - 1)()
            kcs[7]()
            v_hi_chunk(7)()
            v_hi_chunk(8)()   # psum -> v_all copies

            # phase 2: lo halves + x-lo chunks; k-lo paces the chunks, then
            # q0/q1 hi, then the v lo-groups and remaining q terms
            nc.sync.dma_start(wk_t[:, 1, 0:2], wk8[:, 1, 0:2])
            nc.sync.dma_start(cos_sb[:, 0, :], cos[:, 0:TB])
            nc.sync.dma_start(sin_sb[:, 0, :], sin[:, 0:TB])
            nc.sync.dma_start(tri_sb[:], tri[:])
            q0his = q0los = q1his = q1los = None
            for g in range(8):
                _xdma(xl0, xlo, g)
                if g == 0:
                    nc.sync.dma_start(wk_t[:, 1, 2:], wk8[:, 1, 2:])
                    load_wq(0, 0)
                if g == 1:
                    load_wq(1, 0)
                    q0his, q0los = qchunks(0, 0, split=True)
                    q1his, q1los = qchunks(1, 0, split=True)
                if g == 6:
                    nc.sync.dma_start(wv_t[:, 1], wv8[:, 1])
                if g >= 1:
                    kcs[8 + g - 1]()
            kcs[15]()
            for ch in q0his:
                ch()
            for ch in q1his:
                ch()
            kv_finish_k(0, psk0)
            v_lo(0)
            for ch in q0los[:4]:
                ch()
            v_lo(1)
            for ch in q0los[4:]:
                ch()
            v_lo(2)
            for ch in q1los[:4]:
                ch()
            v_lo(3)
            for ch in q1los[4:]:
                ch()

            kv_state = {}

            def mk_rope(j, qts, tb):
                qts[j] = qtp.tile([P, TB], fp16, tag="qt", name=f"qt{j}")

                def tail():
                    rope(qts[j], psqs[j], tb)
                return tail

            for tb in range(NTB):
                enc_hi = [encp.tile([P, 2, TB], fp8, tag=f"ehi{pr}",
                                    name=f"ehi{pr}") for pr in range(4)]
                enc_lo = [encp.tile([P, 2, TB], fp8, tag=f"elo{pr}",
                                    name=f"elo{pr}") for pr in range(4)]
                qts = {}

                # heads 0 and 1: dense q-passes up front (depth-2 priming);
                # rope(0) after both passes, rope(1) deferred into attn(0).
                # For tb 0 the q-projections were emitted in the prologue.
                if tb == 0:
                    load_wq(2, tb)
                else:
                    for ch in qchunks(0, tb):
                        ch()
                    for ch in qchunks(1, tb):
                        ch()
                mk_rope(0, qts, tb)()
                rope1_fn = mk_rope(1, qts, tb)

                nxt = tb + 1 if tb + 1 < NTB else None
                fin = None
                pp = {}

                def pp_mm(tci, pa, pb, pp=pp):
                    def emit():
                        if tci not in pp:
                            pp[tci] = ps.tile([P, TB], f32, tag="big",
                                              bufs=6, name=f"ypsP{tci}")
                        for pr in range(pa, pb):
                            oproj_pair(pp[tci][:], pr, 0, tci,
                                       start=pr == 0, stop=False)
                    return emit

                for h in range(NQ):
                    if h + 3 < NQ and (tb == 0 or h >= 1):
                        load_wq(h + 3, tb)
                    tail_fn = None
                    pe_fills = True
                    if h + 2 < NQ:
                        fills = qchunks(h + 2, tb)
                        tail_fn = mk_rope(h + 2, qts, tb)
                        if nxt is not None and h == 4:
                            # interleave next t-block's x prefetch
                            x_tiles[nxt] = (
                                xp.tile([P, DT, TB], fp8, tag="xh",
                                        name=f"xh{nxt}"),
                                xp.tile([P, DT, TB], fp8, tag="xl",
                                        name=f"xl{nxt}"))
                            xcl = x_dma_closures(nxt, *x_tiles[nxt], nch=4)
                            merged = []
                            for i in range(max(len(fills), len(xcl))):
                                if i < len(fills):
                                    merged.append(fills[i])
                                if i < len(xcl):
                                    merged.append(xcl[i])
                            fills = merged
                        elif nxt is not None and h == 5:
                            fills = fills + [
                                lambda tb=tb: load_wq(0, tb + 1),
                                lambda tb=tb: load_wq(1, tb + 1)]
                    elif h == NQ - 2:
                        # next t-block's k projections + first wo loads
                        fills = [lambda dc=dc: load_wo(dc) for dc in range(2)]
                        if nxt is not None:
                            kc, psk_n = kv_k_chunks(nxt)
                            kv_state["psk"] = psk_n
                            fills = kc + fills
                        else:
                            # last t-block: pre-accumulate pairs 0..2 of
                            # ph3's first d-chunk to shorten the tail
                            fills += [pp_mm(0, 0, 2), pp_mm(1, 0, 2),
                                      pp_mm(0, 2, 3), pp_mm(1, 2, 3)]
                    else:
                        # last head: wo + next wq2 prefetch, then next
                        # t-block's v projections, k rope, v copies
                        fills = [lambda: load_wo(2), lambda: load_wo(3)]
                        if nxt is not None:
                            fills.append(lambda: load_wq(2, nxt))
                            vc, _ = kv_v_chunks(nxt)
                            fills += ([vc[0],
                                       lambda: kv_finish_k(nxt,
                                                           kv_state["psk"])] +
                                      vc[1:])
                        else:
                            pe_fills = False
                    fin = attn(h, tb, qts[h], h // 2, fills, tail_fn,
                               pe_fills, head_fn=rope1_fn if h == 0 else None,
                               fin_prev=fin)
                fin()
                ph3(tb, nxt, pp)

    nc.compile()
    return nc


def _get_nc():
    if "nc" not in _STATE:
        _STATE["nc"] = _build_nc()
    return _STATE["nc"]


def _q8(a):
    return np.ascontiguousarray(a, dtype=np.float32).astype(F8)


def _hilo(a):
    hi = _q8(a)
    lo = _q8(np.asarray(a, np.float32) - hi.astype(np.float32))
    return hi, lo


def _make_in_maps(x, positions, wq, wkv, wo):
    """Build the 8 per-core input dicts (host-side quantization + tables)."""
    B = x.shape[0]
    in_maps = []

    tables = []
    for b in range(B):
        pos = np.asarray(positions[b], np.float64)
        timescale = 10000.0 ** ((2.0 / H) * np.arange(H // 2))
        rad = pos[:, None] / timescale[None, :]          # [T, H/2]
        c64 = np.cos(rad).T                              # [H/2, T]
        s64 = np.sin(rad).T
        tables.append((
            np.ascontiguousarray(np.concatenate([c64, c64], 0)).astype(F16),
            np.ascontiguousarray(np.concatenate([-s64, s64], 0)).astype(F16),
        ))

    xThilo = [_hilo(np.ascontiguousarray(x[b].T)) for b in range(B)]

    i = np.arange(P)[:, None]
    j = np.arange(P)[None, :]
    tri = np.ascontiguousarray((j >= i).astype(F16))

    for c in range(8):
        b, hg = c // 4, c % 4
        qs = slice(NQ * hg, NQ * (hg + 1))
        ks = slice(NKV * hg, NKV * (hg + 1))
        cos_t, sin_t = tables[b]
        xh, xl = xThilo[b]
        # wq8[h]: [P, 2(hi/lo), NDP, 2(pair-half), H], partition-major
        wq_h, wq_l = _hilo(SW_QK * wq[qs])                  # [8, D, H]
        wq8 = np.ascontiguousarray(
            np.stack([wq_h, wq_l], 1)
            .reshape(NQ, 2, NDP, 2, P, H).transpose(0, 4, 1, 2, 3, 5))
        wk_h, wk_l = _hilo(SW_QK * wkv[0, ks].transpose(1, 0, 2))
        wk8 = np.ascontiguousarray(
            np.stack([wk_h, wk_l], 0)
            .reshape(2, NDP, 2, P, NKV, H).transpose(3, 0, 1, 2, 4, 5))
        wv_h, wv_l = _hilo(SW_V * wkv[1, ks].transpose(1, 0, 2))
        wv8 = np.ascontiguousarray(
            np.stack([wv_h, wv_l], 0)
            .reshape(2, NDP, 2, P, NKV * H).transpose(3, 0, 1, 2, 4))
        # wo: [8, H, D] -> [H, 8, D]; adjacent heads form DoubleRow pairs
        wo_h, wo_l = _hilo((SW_O * wo[qs]).transpose(1, 0, 2))
        in_maps.append({
            "xhi": xh,
            "xlo": xl,
            "wq8": wq8,
            "wk8": wk8,
            "wv8": wv8,
            "woh": wo_h,
            "wol": wo_l,
            "cos": cos_t,
            "sin": sin_t,
            "tri": tri,
        })
    return in_maps


def run_cores(in_maps, trace=False, trace_cores=None):
    from concourse.bass_utils import run_bass_kernel_spmd
    nc = _get_nc()
    kw = {}
    if trace:
        kw = dict(trace=True,
                  trace_cores=trace_cores or list(range(8)))
    return run_bass_kernel_spmd(nc, in_maps, core_ids=list(range(8)), **kw)


def kernel(**inputs):
    x = np.asarray(inputs["x"], np.float32)
    positions = np.asarray(inputs["positions"])
    wq = np.asarray(inputs["wq"], np.float32)
    wkv = np.asarray(inputs["wkv"], np.float32)
    wo = np.asarray(inputs["wo"], np.float32)
    B = x.shape[0]
    assert x.shape == (2, T, D) and wq.shape == (32, D, H)

    in_maps = _make_in_maps(x, positions, wq, wkv, wo)
    res = run_cores(in_maps)
    y = np.zeros((B, T, D), np.float32)
    inv = 1.0 / (SW_V * SW_O)
    for c, r in enumerate(res.results):
        y[c // 4] += np.asarray(r["y"], np.float32) * inv
    return y


if __name__ == "__main__":
    _build_nc()
    print("build OK")


# revision 66
# speedup vs baseline: 1.0064x; 1.0064x over previous
"""Trainium2 Bass kernel for GQA attention prefill (B=2, T=2048, D=4096, N=32, K=8, H=128).

Sharding: 8 cores = 2 (batch) x 4 (head-groups). Each core handles one batch
element, 8 q-heads and its 2 kv-heads, producing a partial output projection
(summed over its heads). Host sums the 4 partials per batch element (and
undoes the x512 weight scaling).

Precision scheme (PE cost model: bf16/fp16 1.0 cycles/row, fp8+DoubleRow 0.5
cycles/row with a 256-deep contraction -> 4x effective throughput):
  - q/k/v/o projections run as fp8 DoubleRow with hi+lo error compensation:
    w ~ whi + wlo, x ~ xhi + xlo (each e4m3), y = whi@xhi + wlo@xhi + whi@xlo.
    3 quarter-cost matmuls = 0.75x the bf16 cost at ~0.1% error. Weights are
    pre-scaled into e4m3's normal range (wq,wk x64 folded into the exp scale;
    wv x16 cancels against the softmax 1/l fold; wo x32 undone on host).
  - one o-proj head-pair runs direct fp8 (1 matmul, 0.25x cost), spending the
    correctness headroom (~1.7% of final norm).
  - attention (rope, logits, exp, AV) runs in fp16: same PE cost as bf16,
    ~8x lower noise.

Per-core pipeline, software-pipelined per head so PE never idles:
  passA(tb):  k,v projections from xhi/xlo (DMA'd once per t-block, resident
              in SBUF); rope(k) via SBUF->SBUF DMA half-swap plus DVE
              elementwise with fp16 cos/sin tables ([-sin; sin] fold).
  per head h: q-projection matmuls for head h+2 are emitted interleaved with
              head h's attention s-block loop. Attention: logitsT [s128,t<=512]
              = kT-block @ qt (fp16), exp on ACT (scale absorbs the x64 weight
              scales), 0/1 triangle mask multiply on DVE for diagonal tiles,
              AV accumulates in PSUM; denominators accumulate on DVE in f32.
  fin(h):     gpsimd partition reduce -> reciprocal -> DVE psum*rinv -> f32
              tmp, then ACT copy -> enc_hi (fp8) and DVE sub -> enc_lo (fp8),
              pair-interleaved for the o-proj stationary operand.
  ph3(tb):    output projection from enc pair tiles: 3 pairs x 3-term + 1
              direct pair = 10 DoubleRow matmuls per (dchunk, tchunk); PSUM ->
              bf16 SBUF copies on ACT, DMA out per 512-wide d-chunk.
"""

import os
import sys

import numpy as np

for _p in ("/opt/trn_rl_repo", "/root/.axon_site/_ro/trn_rl_repo"):
    if _p not in sys.path and os.path.isdir(_p):
        sys.path.append(_p)

import ml_dtypes

BF16 = ml_dtypes.bfloat16
F16 = np.float16
F8 = ml_dtypes.float8_e4m3fn

P = 128
T = 2048
D = 4096
H = 128
NQ = 8   # q heads per core
NKV = 2  # kv heads per core
TB = 512
NTB = T // TB        # 4
DT = D // P          # 32 d-tiles
NDP = DT // 2        # 16 d-tile pairs
NSB = T // P         # 16 s-blocks
TC = TB // P         # 4 t-chunks per t-block
NDC = D // TB        # 8 d-chunks for the output projection
SCALE = float(H) ** -0.5
SW_QK = 64.0         # wq/wk host scale (folded into exp scale)
SW_V = 16.0          # wv host scale (cancels vs softmax 1/l fold)
SW_O = 32.0          # wo host scale (undone on host with 1/(SW_V*SW_O))
EXP_SCALE = SCALE / (SW_QK * SW_QK)
DIRECT_PAIR = 3      # o-proj head pair computed hi@hi only ...
DIRECT_DCS = frozenset(range(NDC))  # ... on these 512-wide d-chunks

_STATE = {}


def _build_nc():
    import concourse.mybir as mybir
    import concourse.tile as tile
    from concourse import bacc
    from concourse import bass_isa

    f32 = mybir.dt.float32
    fp16 = mybir.dt.float16
    fp8 = mybir.dt.float8e4
    bf16 = mybir.dt.bfloat16
    Alu = mybir.AluOpType
    Act = mybir.ActivationFunctionType
    DR = mybir.MatmulPerfMode.DoubleRow

    nc = bacc.Bacc(None, target_bir_lowering=False, debug=False)

    xhi = nc.dram_tensor("xhi", [D, T], fp8, kind="ExternalInput")
    xlo = nc.dram_tensor("xlo", [D, T], fp8, kind="ExternalInput")
    # weights are partition-major and hi/lo-packed so each load is one
    # fully-contiguous DMA (>=512B runs avoid the half-bandwidth penalty):
    # wq8[h, p] = [NDP, 2(hi/lo), 2(pair), H], wk8/wv8[p] likewise
    wq8 = nc.dram_tensor("wq8", [NQ, P, 2, NDP, 2, H], fp8,
                         kind="ExternalInput")
    wk8 = nc.dram_tensor("wk8", [P, 2, NDP, 2, NKV, H], fp8,
                         kind="ExternalInput")
    wv8 = nc.dram_tensor("wv8", [P, 2, NDP, 2, NKV * H], fp8,
                         kind="ExternalInput")
    # o-projection weights, head-major within rows: [H, NQ, D] (adjacent
    # heads form the DoubleRow pairs)
    woh = nc.dram_tensor("woh", [H, NQ, D], fp8, kind="ExternalInput")
    wol = nc.dram_tensor("wol", [H, NQ, D], fp8, kind="ExternalInput")
    cos = nc.dram_tensor("cos", [P, T], fp16, kind="ExternalInput")
    sin = nc.dram_tensor("sin", [P, T], fp16, kind="ExternalInput")
    tri = nc.dram_tensor("tri", [P, P], fp16, kind="ExternalInput")
    y = nc.dram_tensor("y", [T, D], bf16, kind="ExternalOutput")

    with tile.TileContext(nc) as tc:
        with (
            tc.tile_pool(name="const", bufs=1) as const,
            tc.tile_pool(name="xp", bufs=2) as xp,
            tc.tile_pool(name="wqp", bufs=3) as wqp,
            tc.tile_pool(name="qtp", bufs=3) as qtp,
            tc.tile_pool(name="rp", bufs=2) as rp,
            tc.tile_pool(name="ep", bufs=6) as ep,
            tc.tile_pool(name="eap", bufs=2) as eap,
            tc.tile_pool(name="encp", bufs=1) as encp,
            tc.tile_pool(name="lp", bufs=1) as lp,
            tc.tile_pool(name="wop", bufs=2) as wop,
            tc.tile_pool(name="yp", bufs=2) as yp,
            tc.tile_pool(name="ps", bufs=1, space="PSUM") as ps,
        ):
            wk_t = const.tile([P, 2, NDP, 2, NKV, H], fp8, tag="wk")
            wv_t = const.tile([P, 2, NDP, 2, NKV * H], fp8, tag="wv")
            tri_sb = const.tile([P, P], fp16, tag="tri")
            cos_sb = const.tile([P, 2, TB], fp16, tag="cos")
            sin_sb = const.tile([P, 2, TB], fp16, tag="sin")
            kT_all = const.tile([P, NKV, T], fp16, tag="kT")
            v_all = const.tile([P, NKV, NSB, H], fp16, tag="v")

            def x_dma_closures(tb, x_hi, x_lo, tables=True, nch=8):
                """nch closures, each DMA-ing a DT/nch-d-tile chunk of
                xhi+xlo for tb (plus this t-block's cos/sin on chunk 0)."""
                tsl = slice(tb * TB, (tb + 1) * TB)
                step = DT // nch

                def mk(c8):
                    def emit():
                        dsl = slice(c8 * step * P, (c8 + 1) * step * P)
                        csl = slice(c8 * step, (c8 + 1) * step)
                        nc.sync.dma_start(
                            x_hi[:, csl, :],
                            xhi[dsl, tsl].rearrange("(g p) t -> p g t", p=P))
                        nc.sync.dma_start(
                            x_lo[:, csl, :],
                            xlo[dsl, tsl].rearrange("(g p) t -> p g t", p=P))
                        if c8 == 0 and tables:
                            nc.sync.dma_start(cos_sb[:, tb % 2, :],
                                              cos[:, tsl])
                            nc.sync.dma_start(sin_sb[:, tb % 2, :],
                                              sin[:, tsl])
                    return emit

                return [mk(c8) for c8 in range(nch)]

            def mm3(out, wf, mf, pr, start, stop):
                """3-term compensated DoubleRow accumulation for d-pair pr.
                wf(pr, s) -> [P, 2, M] stationary slice, mf(pr, s) ->
                [P, 2, N] moving slice (s: 0=hi, 1=lo)."""
                nc.tensor.matmul(out, wf(pr, 0), mf(pr, 0),
                                 start=start, stop=False, perf_mode=DR)
                nc.tensor.matmul(out, wf(pr, 1), mf(pr, 0),
                                 start=False, stop=False, perf_mode=DR)
                nc.tensor.matmul(out, wf(pr, 0), mf(pr, 1),
                                 start=False, stop=stop, perf_mode=DR)

            def rope(dst, src_ps, tb):
                """dst[:] = rope(src_ps) for one head's [H, TB] block (fp16).
                Half-swap via PSUM->SBUF DMA partition reorder; the sign of
                the swapped half is folded into the sin table ([-sin; +sin])."""
                cs = cos_sb[:, tb % 2, :]
                sn = sin_sb[:, tb % 2, :]
                raw = rp.tile([P, TB], fp16, tag="raw", name="raw")
                nc.scalar.copy(raw[:], src_ps[:])
                shuf = rp.tile([P, TB], fp16, tag="shuf", name="shuf")
                nc.sync.dma_start(shuf[0:P // 2, :], raw[P // 2:P, :])
                nc.sync.dma_start(shuf[P // 2:P, :], raw[0:P // 2, :])
                tmp = rp.tile([P, TB], fp16, tag="tmp", name="tmp")
                nc.vector.tensor_tensor(dst, raw[:], cs, Alu.mult)
                nc.vector.tensor_tensor(tmp[:], shuf[:], sn, Alu.mult)
                nc.vector.tensor_tensor(dst, dst, tmp[:], Alu.add)

            def kv_k_chunks(tbx, split=False):
                """k-projection chunk closures (+ psk tiles) for tbx. With
                split=True, returns hi-closures (hi@hi terms only) followed
                by lo-closures (compensation terms), so the prologue can
                start on the hi DMAs alone."""
                x_hi, x_lo = x_tiles[tbx]
                psk = [ps.tile([P, TB], f32, tag="big", bufs=6,
                               name=f"psk{_k}") for _k in range(NKV)]

                def xf(pr, s):
                    return (x_hi if s == 0 else x_lo)[:, 2 * pr:2 * pr + 2, :]

                def wf(kk):
                    return lambda pr, s: wk_t[:, s, pr, :, kk, :]

                def mk(p0, p1):
                    def emit():
                        for pr in range(p0, p1):
                            for kk in range(NKV):
                                mm3(psk[kk][:], wf(kk), xf, pr,
                                    start=pr == 0, stop=pr == NDP - 1)
                    return emit

                def mk_hi(p0, p1):
                    def emit():
                        for pr in range(p0, p1):
                            for kk in range(NKV):
                                nc.tensor.matmul(
                                    psk[kk][:], wk_t[:, 0, pr, :, kk, :],
                                    xf(pr, 0), start=pr == 0,
                                    stop=False, perf_mode=DR)
                    return emit

                def mk_lo(p0, p1):
                    def emit():
                        for pr in range(p0, p1):
                            for kk in range(NKV):
                                nc.tensor.matmul(
                                    psk[kk][:], wk_t[:, 1, pr, :, kk, :],
                                    xf(pr, 0), start=False,
                                    stop=False, perf_mode=DR)
                                nc.tensor.matmul(
                                    psk[kk][:], wk_t[:, 0, pr, :, kk, :],
                                    xf(pr, 1), start=False,
                                    stop=pr == NDP - 1 and kk == NKV - 1,
                                    perf_mode=DR)
                    return emit

                if split:
                    cls = ([mk_hi(p, p + 2) for p in range(0, NDP, 2)] +
                           [mk_lo(p, p + 2) for p in range(0, NDP, 2)])
                    return cls, psk
                return [mk(p, p + 2) for p in range(0, NDP, 2)], psk

            def kv_v_chunks(tbx):
                """v-projection closures for tbx. Each of the 4 t-chunks gets
                its OWN PSUM tile, sequenced so a chunk is copied out to
                v_all before its bank slot is reused."""
                x_hi, x_lo = x_tiles[tbx]
                state = {}

                def mk_mm(c, p0, p1):
                    def emit():
                        if p0 == 0:
                            state[c] = ps.tile([P, NKV * H], f32, tag="big",
                                               bufs=6, name=f"psv{c}")
                        csl = slice(c * P, (c + 1) * P)

                        def xf(pr, s):
                            return (x_hi if s == 0
                                    else x_lo)[:, 2 * pr:2 * pr + 2, csl]

                        for pr in range(p0, p1):
                            mm3(state[c][:], xf,
                                lambda pr, s: wv_t[:, s, pr, :, :], pr,
                                start=pr == 0, stop=pr == NDP - 1)
                    return emit

                def mk_copy(c):
                    def emit():
                        nc.scalar.copy(
                            v_all[:, :, tbx * TC + c, :],
                            state[c][:].rearrange("p (h e) -> p h e", h=NKV))
                    return emit

                chunks = []
                for c in (0, 2):
                    chunks += [mk_mm(c, p, p + 4) for p in range(0, NDP, 4)]
                chunks.append(mk_copy(0))
                chunks += [mk_mm(1, p, p + 4) for p in range(0, NDP, 4)]
                chunks.append(mk_copy(2))
                chunks += [mk_mm(3, p, p + 4) for p in range(0, NDP, 4)]
                chunks.append(mk_copy(1))
                chunks.append(mk_copy(3))
                return chunks, None

            def kv_finish_k(tbx, psk):
                tsl = slice(tbx * TB, (tbx + 1) * TB)
                for kk in range(NKV):
                    rope(kT_all[:, kk, tsl], psk[kk], tbx)

            wqs = {}
            psqs = {}

            def load_wq(h, tb):
                wq_t = wqp.tile([P, 2, NDP, 2, H], fp8, tag="wq",
                                name=f"wq{h}")
                nc.sync.dma_start(wq_t[:], wq8[h])
                wqs[h] = wq_t

            def qchunks(h, tb, split=False):
                """Closures each emitting a few of head h's 48 accumulating
                q-projection DoubleRow matmuls (wq must already be loading)."""
                x_hi, x_lo = x_tiles[tb]
                wq_t = wqs[h]
                psq = ps.tile([P, TB], f32, tag="big", bufs=6, name=f"psq{h}")
                psqs[h] = psq

                def xf(pr, s):
                    return (x_hi if s == 0 else x_lo)[:, 2 * pr:2 * pr + 2, :]

                def mk(p0, p1):
                    def emit():
                        for pr in range(p0, p1):
                            mm3(psq[:], lambda pr, s: wq_t[:, s, pr, :, :],
                                xf, pr, start=pr == 0, stop=pr == NDP - 1)
                    return emit

                if split:
                    def mk_hi(p0, p1):
                        def emit():
                            for pr in range(p0, p1):
                                nc.tensor.matmul(
                                    psq[:], wq_t[:, 0, pr, :, :], xf(pr, 0),
                                    start=pr == 0, stop=False, perf_mode=DR)
                        return emit

                    def mk_lo(p0, p1):
                        def emit():
                            for pr in range(p0, p1):
                                nc.tensor.matmul(
                                    psq[:], wq_t[:, 1, pr, :, :], xf(pr, 0),
                                    start=False, stop=False, perf_mode=DR)
                                nc.tensor.matmul(
                                    psq[:], wq_t[:, 0, pr, :, :], xf(pr, 1),
                                    start=False, stop=pr == NDP - 1,
                                    perf_mode=DR)
                        return emit

                    return ([mk_hi(p, p + 2) for p in range(0, NDP, 2)],
                            [mk_lo(p, p + 2) for p in range(0, NDP, 2)])
                return [mk(p, p + 2) for p in range(0, NDP, 2)]

            def attn(h, tb, qt, hp, fills, tail_fn, pe_fills=True,
                     head_fn=None, fin_prev=None):
                """Attention for head h. `fills` are closures interleaved into
                the s-block loop (a later head's q matmuls, or DMA prefetch);
                `tail_fn` (that head's rope) is emitted once fills are done."""
                nsb = TC * (tb + 1)
                ndg = nsb - TC  # non-diagonal s-block count
                kk = h // 4
                enc_ps = ps.tile([P, TB], f32, tag="acc", bufs=2,
                                 name=f"encps{h}")
                exacc = eap.tile([P, TB], fp16, tag="eacc", name=f"eacc{h}")
                look = 5
                ex = [None] * nsb
                tail_state = {"done": tail_fn is None}

                def pre(sb):
                    r = sb - ndg
                    off = P * r if r >= 0 else 0
                    csl = slice(off, TB)
                    lg = ps.tile([P, TB], f32, tag="big", bufs=6,
                                 name=f"lg{sb}")
                    nc.tensor.matmul(lg[:, csl],
                                     kT_all[:, kk, sb * P:(sb + 1) * P],
                                     qt[:, csl])
                    ex_t = ep.tile([P, TB], fp16, tag="ex", name=f"ex{sb}")
                    nc.scalar.activation(ex_t[:, csl], lg[:, csl], Act.Exp,
                                         scale=EXP_SCALE)
                    if r >= 0:
                        nc.vector.tensor_tensor(ex_t[:, off:off + P],
                                                ex_t[:, off:off + P],
                                                tri_sb[:], Alu.mult)
                    ex[sb] = (ex_t, csl)

                def post(sb):
                    ex_t, csl = ex[sb]
                    nc.tensor.matmul(enc_ps[:, csl], v_all[:, kk, sb, :],
                                     ex_t[:, csl],
                                     start=sb == 0, stop=sb == nsb - 1)
                    # exp-tile accumulation for the softmax denominator (DVE)
                    if sb == 0:
                        nc.vector.tensor_copy(exacc[:], ex_t[:])
                    else:
                        nc.vector.tensor_tensor(exacc[:, csl], exacc[:, csl],
                                                ex_t[:, csl], Alu.add)

                ci = 0
                while ci < min(2, len(fills)):
                    fills[ci]()
                    ci += 1
                if head_fn is not None:
                    head_fn()
                for sb in range(min(look, nsb)):
                    pre(sb)
                if fin_prev is not None:
                    fin_prev()
                for sb in range(nsb):
                    want = min(len(fills),
                               max((sb + 3) * len(fills) // nsb, 3))
                    while ci < want:
                        fills[ci]()
                        ci += 1
                    if ci == len(fills) and not tail_state["done"]:
                        tail_fn()
                        tail_state["done"] = True
                    post(sb)
                    if sb + look < nsb:
                        pre(sb + look)
                while ci < len(fills):
                    fills[ci]()
                    ci += 1
                if not tail_state["done"]:
                    tail_fn()

                def finalize():
                    lsum = lp.tile([P, TB], f32, tag="lsum", name="lsum")
                    nc.gpsimd.partition_all_reduce(lsum[:], exacc[:], P,
                                                   bass_isa.ReduceOp.add)
                    rinv = lp.tile([P, TB], f32, tag="rinv", name="rinv")
                    nc.vector.reciprocal(rinv[:], lsum[:])
                    tmp = lp.tile([P, TB], f32, tag="etmp", name="etmp")
                    nc.vector.tensor_tensor(tmp[:], enc_ps[:], rinv[:],
                                            Alu.mult)
                    pr, parity = h // 2, h % 2
                    hi_sl = enc_hi[pr][:, parity, :]
                    nc.vector.tensor_copy(hi_sl, tmp[:])
                    if pr != DIRECT_PAIR:
                        nc.vector.tensor_tensor(enc_lo[pr][:, parity, :],
                                                tmp[:], hi_sl, Alu.subtract)
                return finalize

            wo_tiles = {}

            def load_wo(dc):
                dsl = slice(dc * TB, (dc + 1) * TB)
                wo_h = wop.tile([P, NQ, TB], fp8, tag="woh", bufs=4,
                                name=f"woh{dc}")
                nc.sync.dma_start(wo_h[:], woh[:, :, dsl])
                wo_l = wop.tile([P, 6, TB], fp8, tag="wol", bufs=4,
                                name=f"wol{dc}")
                nc.sync.dma_start(wo_l[:], wol[:, 0:6, dsl])
                wo_l2 = None
                if dc not in DIRECT_DCS:
                    # pair 3's lo weights are only read on non-direct chunks
                    wo_l2 = wop.tile([P, 2, TB], fp8, tag="wol2", bufs=1,
                                     name=f"wol2_{dc}")
                    nc.sync.dma_start(wo_l2[:], wol[:, 6:8, dsl])
                wo_tiles[dc] = (wo_h, wo_l, wo_l2)

            def oproj_pair(yps, pr, dc, tci, start, stop):
                """o-projection matmuls for head pair pr into yps.
                3-term compensated, or single hi@hi for the direct pair on
                the direct d-chunks."""
                wo_h, wo_l, wo_l2 = wo_tiles[dc]
                tsl = slice(tci * P, (tci + 1) * P)
                psl = slice(2 * pr, 2 * pr + 2)
                e_hi = enc_hi[pr][:, :, tsl]
                direct = pr == DIRECT_PAIR and dc in DIRECT_DCS
                nc.tensor.matmul(yps, e_hi, wo_h[:, psl, :], start=start,
                                 stop=stop and direct, perf_mode=DR)
                if not direct:
                    lo_sl = (wo_l2[:, 0:2, :] if pr == DIRECT_PAIR
                             else wo_l[:, psl, :])
                    nc.tensor.matmul(yps, e_hi, lo_sl, start=False,
                                     stop=False, perf_mode=DR)
                    nc.tensor.matmul(yps, enc_lo[pr][:, :, tsl],
                                     wo_h[:, psl, :], start=False, stop=stop,
                                     perf_mode=DR)

            def ph3(tb, nxt, pp=None):
                for dc in range(NDC):
                    if nxt is not None and dc == 4:
                        load_wq(3, nxt)
                    if dc + 4 < NDC:
                        load_wo(dc + 4)
                    # On dc 0, pair 3 (DIRECT_PAIR, whose enc depends on the
                    # final fin chain) is deferred behind the other pairs so
                    # fin's latency hides behind PE work; later dcs complete
                    # each t-chunk immediately so its PSUM bank frees early.
                    def finish(tci, yps):
                        oproj_pair(yps[:], DIRECT_PAIR, dc, tci,
                                   start=False, stop=True)
                        ys = yp.tile([P, TB], bf16, tag="ys", bufs=5,
                                     name=f"ys{dc}_{tci}")
                        if (dc * TC + tci) % 2 == 0:
                            nc.scalar.copy(ys[:], yps[:])
                        else:
                            nc.vector.tensor_copy(ys[:], yps[:])
                        nc.sync.dma_start(
                            y[tb * TB + tci * P:tb * TB + (tci + 1) * P,
                              dc * TB:(dc + 1) * TB], ys[:])

                    ytiles = {}
                    for tci in range(TC):
                        if pp and dc == 0 and tci in pp:
                            ytiles[tci] = pp[tci]  # pairs 0..2 already done
                            continue
                        yps = ps.tile([P, TB], f32, tag="big", bufs=6,
                                      name=f"yps{dc}_{tci}")
                        ytiles[tci] = yps
                        for pr in range(3):
                            oproj_pair(yps[:], pr, dc, tci,
                                       start=pr == 0, stop=False)
                        if dc != 0:
                            finish(tci, yps)
                    if dc == 0:
                        for tci in range(TC):
                            finish(tci, ytiles[tci])

            # ---- startup: hi-phase first (k hi@hi streams on the hi DMAs
            # alone), then lo/v phase ----
            xh0 = xp.tile([P, DT, TB], fp8, tag="xh", name="xh0")
            xl0 = xp.tile([P, DT, TB], fp8, tag="xl", name="xl0")
            x_tiles = {0: (xh0, xl0)}

            def _xdma(dst, src, g):
                nc.sync.dma_start(
                    dst[:, 4 * g:4 * (g + 1), :],
                    src[4 * g * P:4 * (g + 1) * P, 0:TB]
                    .rearrange("(g p) t -> p g t", p=P))

            kcs, psk0 = kv_k_chunks(0, split=True)  # 8 hi + 8 lo closures

            # v-projection runs as two PSUM groups in the prologue: the hi
            # group completes during the hi-DMA phase (PE fill), the lo group
            # in phase 2, DVE-added into v_all afterwards.
            vhi_state = {}
            vlo_state = {}

            def v_hi_chunk(g):
                def emit():
                    for c in range(TC):
                        if g == 0:
                            vhi_state[c] = ps.tile([P, NKV * H], f32,
                                                   tag="big", bufs=6,
                                                   name=f"psvh{c}")
                        csl = slice(c * P, (c + 1) * P)
                        for pr in (2 * g, 2 * g + 1):
                            nc.tensor.matmul(
                                vhi_state[c][:],
                                xh0[:, 2 * pr:2 * pr + 2, csl],
                                wv_t[:, 0, pr, :, :],
                                start=pr == 0, stop=pr == NDP - 1,
                                perf_mode=DR)
                def v_hi_copy():
                    for c in range(TC):
                        nc.scalar.copy(
                            v_all[:, :, c, :],
                            vhi_state[c][:]
                            .rearrange("p (h e) -> p h e", h=NKV))
                return emit if g < 8 else v_hi_copy

            def v_lo(c):
                vlo_state[c] = ps.tile([P, NKV * H], f32, tag="big", bufs=6,
                                       name=f"psvl{c}")
                csl = slice(c * P, (c + 1) * P)
                for pr in range(NDP):
                    nc.tensor.matmul(vlo_state[c][:],
                                     xl0[:, 2 * pr:2 * pr + 2, csl],
                                     wv_t[:, 0, pr, :, :],
                                     start=pr == 0, stop=False,
                                     perf_mode=DR)
                    nc.tensor.matmul(vlo_state[c][:],
                                     xh0[:, 2 * pr:2 * pr + 2, csl],
                                     wv_t[:, 1, pr, :, :],
                                     start=False, stop=pr == NDP - 1,
                                     perf_mode=DR)
                nc.vector.tensor_tensor(
                    v_all[:, :, c, :], v_all[:, :, c, :],
                    vlo_state[c][:].rearrange("p (h e) -> p h e", h=NKV),
                    Alu.add)

            # phase 1: hi halves of wk/wv (first pair-chunk split off so PE
            # can start early) + x-hi chunks, with k-hi and v-hi matmuls
            # pacing behind the chunks
            nc.sync.dma_start(wk_t[:, 0, 0:2], wk8[:, 0, 0:2])
            _xdma(xh0, xhi, 0)
            nc.sync.dma_start(wv_t[:, 0, 0:2], wv8[:, 0, 0:2])
            nc.sync.dma_start(wk_t[:, 0, 2:], wk8[:, 0, 2:])
            nc.sync.dma_start(wv_t[:, 0, 2:], wv8[:, 0, 2:])
            for g in range(1, 8):
                _xdma(xh0, xhi, g)
                kcs[g - 1]()
                v_hi_chunk(g - 1)()
            kcs[7]()
            v_hi_chunk(7)()
            v_hi_chunk(8)()   # psum -> v_all copies

            # phase 2: lo halves + x-lo chunks; k-lo paces the chunks, then
            # q0/q1 hi, then the v lo-groups and remaining q terms
            nc.sync.dma_start(wk_t[:, 1, 0:2], wk8[:, 1, 0:2])
            nc.sync.dma_start(cos_sb[:, 0, :], cos[:, 0:TB])
            nc.sync.dma_start(sin_sb[:, 0, :], sin[:, 0:TB])
            nc.sync.dma_start(tri_sb[:], tri[:])
            q0his = q0los = q1his = q1los = None
            for g in range(8):
                _xdma(xl0, xlo, g)
                if g == 0:
                    nc.sync.dma_start(wk_t[:, 1, 2:], wk8[:, 1, 2:])
                    load_wq(0, 0)
                if g == 1:
                    load_wq(1, 0)
                    q0his, q0los = qchunks(0, 0, split=True)
                    q1his, q1los = qchunks(1, 0, split=True)
                if g == 6:
                    nc.sync.dma_start(wv_t[:, 1], wv8[:, 1])
                if g >= 1:
                    kcs[8 + g - 1]()
            kcs[15]()
            for ch in q0his:
                ch()
            for ch in q1his:
                ch()
            kv_finish_k(0, psk0)
            v_lo(0)
            for ch in q0los[:4]:
                ch()
            v_lo(1)
            for ch in q0los[4:]:
                ch()
            v_lo(2)
            for ch in q1los[:4]:
                ch()
            v_lo(3)
            for ch in q1los[4:]:
                ch()

            kv_state = {}

            def mk_rope(j, qts, tb):
                qts[j] = qtp.tile([P, TB], fp16, tag="qt", name=f"qt{j}")

                def tail():
                    rope(qts[j], psqs[j], tb)
                return tail

            for tb in range(NTB):
                enc_hi = [encp.tile([P, 2, TB], fp8, tag=f"ehi{pr}",
                                    name=f"ehi{pr}") for pr in range(4)]
                enc_lo = [encp.tile([P, 2, TB], fp8, tag=f"elo{pr}",
                                    name=f"elo{pr}")
                          if pr != DIRECT_PAIR else None for pr in range(4)]
                qts = {}

                # heads 0 and 1: dense q-passes up front (depth-2 priming);
                # rope(0) after both passes, rope(1) deferred into attn(0).
                # For tb 0 the q-projections were emitted in the prologue.
                if tb == 0:
                    load_wq(2, tb)
                else:
                    for ch in qchunks(0, tb):
                        ch()
                    for ch in qchunks(1, tb):
                        ch()
                mk_rope(0, qts, tb)()
                rope1_fn = mk_rope(1, qts, tb)

                nxt = tb + 1 if tb + 1 < NTB else None
                fin = None
                pp = {}

                def pp_mm(tci, pa, pb, pp=pp):
                    def emit():
                        if tci not in pp:
                            pp[tci] = ps.tile([P, TB], f32, tag="big",
                                              bufs=6, name=f"ypsP{tci}")
                        for pr in range(pa, pb):
                            oproj_pair(pp[tci][:], pr, 0, tci,
                                       start=pr == 0, stop=False)
                    return emit

                for h in range(NQ):
                    if h + 3 < NQ and (tb == 0 or h >= 1):
                        load_wq(h + 3, tb)
                    tail_fn = None
                    pe_fills = True
                    if h + 2 < NQ:
                        fills = qchunks(h + 2, tb)
                        tail_fn = mk_rope(h + 2, qts, tb)
                        if nxt is not None and h == 4:
                            # interleave next t-block's x prefetch
                            x_tiles[nxt] = (
                                xp.tile([P, DT, TB], fp8, tag="xh",
                                        name=f"xh{nxt}"),
                                xp.tile([P, DT, TB], fp8, tag="xl",
                                        name=f"xl{nxt}"))
                            xcl = x_dma_closures(nxt, *x_tiles[nxt], nch=4)
                            merged = []
                            for i in range(max(len(fills), len(xcl))):
                                if i < len(fills):
                                    merged.append(fills[i])
                                if i < len(xcl):
                                    merged.append(xcl[i])
                            fills = merged
                        elif nxt is not None and h == 5:
                            fills = fills + [
                                lambda tb=tb: load_wq(0, tb + 1),
                                lambda tb=tb: load_wq(1, tb + 1)]
                    elif h == NQ - 2:
                        # next t-block's k projections + first wo loads
                        fills = [lambda dc=dc: load_wo(dc) for dc in range(2)]
                        if nxt is not None:
                            kc, psk_n = kv_k_chunks(nxt)
                            kv_state["psk"] = psk_n
                            fills = kc + fills
                        else:
                            # last t-block: pre-accumulate pairs 0..2 of
                            # ph3's first d-chunk to shorten the tail
                            fills += [pp_mm(0, 0, 2), pp_mm(1, 0, 2),
                                      pp_mm(0, 2, 3), pp_mm(1, 2, 3)]
                    else:
                        # last head: wo + next wq2 prefetch, then next
                        # t-block's v projections, k rope, v copies
                        fills = [lambda: load_wo(2), lambda: load_wo(3)]
                        if nxt is not None:
                            fills.append(lambda: load_wq(2, nxt))
                            vc, _ = kv_v_chunks(nxt)
                            fills += ([vc[0],
                                       lambda: kv_finish_k(nxt,
                                                           kv_state["psk"])] +
                                      vc[1:])
                        else:
                            pe_fills = False
                    fin = attn(h, tb, qts[h], h // 2, fills, tail_fn,
                               pe_fills, head_fn=rope1_fn if h == 0 else None,
                               fin_prev=fin)
                fin()
                ph3(tb, nxt, pp)

    nc.compile()
    return nc


def _get_nc():
    if "nc" not in _STATE:
        _STATE["nc"] = _build_nc()
    return _STATE["nc"]


def _q8(a):
    return np.ascontiguousarray(a, dtype=np.float32).astype(F8)


def _hilo(a):
    hi = _q8(a)
    lo = _q8(np.asarray(a, np.float32) - hi.astype(np.float32))
    return hi, lo


def _make_in_maps(x, positions, wq, wkv, wo):
    """Build the 8 per-core input dicts (host-side quantization + tables)."""
    B = x.shape[0]
    in_maps = []

    tables = []
    for b in range(B):
        pos = np.asarray(positions[b], np.float64)
        timescale = 10000.0 ** ((2.0 / H) * np.arange(H // 2))
        rad = pos[:, None] / timescale[None, :]          # [T, H/2]
        c64 = np.cos(rad).T                              # [H/2, T]
        s64 = np.sin(rad).T
        tables.append((
            np.ascontiguousarray(np.concatenate([c64, c64], 0)).astype(F16),
            np.ascontiguousarray(np.concatenate([-s64, s64], 0)).astype(F16),
        ))

    xThilo = [_hilo(np.ascontiguousarray(x[b].T)) for b in range(B)]

    i = np.arange(P)[:, None]
    j = np.arange(P)[None, :]
    tri = np.ascontiguousarray((j >= i).astype(F16))

    for c in range(8):
        b, hg = c // 4, c % 4
        qs = slice(NQ * hg, NQ * (hg + 1))
        ks = slice(NKV * hg, NKV * (hg + 1))
        cos_t, sin_t = tables[b]
        xh, xl = xThilo[b]
        # wq8[h]: [P, 2(hi/lo), NDP, 2(pair-half), H], partition-major
        wq_h, wq_l = _hilo(SW_QK * wq[qs])                  # [8, D, H]
        wq8 = np.ascontiguousarray(
            np.stack([wq_h, wq_l], 1)
            .reshape(NQ, 2, NDP, 2, P, H).transpose(0, 4, 1, 2, 3, 5))
        wk_h, wk_l = _hilo(SW_QK * wkv[0, ks].transpose(1, 0, 2))
        wk8 = np.ascontiguousarray(
            np.stack([wk_h, wk_l], 0)
            .reshape(2, NDP, 2, P, NKV, H).transpose(3, 0, 1, 2, 4, 5))
        wv_h, wv_l = _hilo(SW_V * wkv[1, ks].transpose(1, 0, 2))
        wv8 = np.ascontiguousarray(
            np.stack([wv_h, wv_l], 0)
            .reshape(2, NDP, 2, P, NKV * H).transpose(3, 0, 1, 2, 4))
        # wo: [8, H, D] -> [H, 8, D]; adjacent heads form DoubleRow pairs
        wo_h, wo_l = _hilo((SW_O * wo[qs]).transpose(1, 0, 2))
        in_maps.append({
            "xhi": xh,
            "xlo": xl,
            "wq8": wq8,
            "wk8": wk8,
            "wv8": wv8,
            "woh": wo_h,
            "wol": wo_l,
            "cos": cos_t,
            "sin": sin_t,
            "tri": tri,
        })
    return in_maps


def run_cores(in_maps, trace=False, trace_cores=None):
    from concourse.bass_utils import run_bass_kernel_spmd
    nc = _get_nc()
    kw = {}
    if trace:
        kw = dict(trace=True,
                  trace_cores=trace_cores or list(range(8)))
    return run_bass_kernel_spmd(nc, in_maps, core_ids=list(range(8)), **kw)


def kernel(**inputs):
    x = np.asarray(inputs["x"], np.float32)
    positions = np.asarray(inputs["positions"])
    wq = np.asarray(inputs["wq"], np.float32)
    wkv = np.asarray(inputs["wkv"], np.float32)
    wo = np.asarray(inputs["wo"], np.float32)
    B = x.shape[0]
    assert x.shape == (2, T, D) and wq.shape == (32, D, H)

    in_maps = _make_in_maps(x, positions, wq, wkv, wo)
    res = run_cores(in_maps)
    y = np.zeros((B, T, D), np.float32)
    inv = 1.0 / (SW_V * SW_O)
    for c, r in enumerate(res.results):
        y[c // 4] += np.asarray(r["y"], np.float32) * inv
    return y


if __name__ == "__main__":
    _build_nc()
    print("build OK")
